# revision 7
# baseline (speedup 1.0000x reference)
"""Self-contained Trainium2 Bass kernel for the 3-layer GCN (AgriGraphGCN).

kernel(**inputs) -> (100000, 1) float32 risk scores, computed SPMD on 8
NeuronCores. Strategy: shard nodes by dst owner; per layer build a bf16
feature table (project + PE-transpose + degree scaling) in PIECES, each
piece AllGather'd as soon as written so cross-core gathers overlap the
collective; dma_gather per-edge source rows (4 SWDGE queues), scatter via
one-hot PE matmuls into PSUM accumulators, BatchNorm from AllReduce'd
masked stats with the affine+relu application fused into the next layer's
projection pass. L3 (scalar messages) scatters single columns.
"""
import sys
sys.path.insert(0, "/opt/trn_rl_repo")

import numpy as np
import ml_dtypes


def make_cfg(full=True):
    if full:
        return dict(N=100000, E=640000, NPC=12500, NL=12544, G=14, IN=6,
                    PIECES=[32, 32, 32, 2])
    return dict(N=4096, E=16384, NPC=512, NL=512, G=2, IN=6, PIECES=[2, 2])


C = 8           # cores
H = 128         # hidden
NI_MAX = 1024   # dma_gather per-call limit
BN_EPS = 1e-5


def host_prep(cfg, edge_index):
    """Build common (cross-core) batch/op structure + per-core index data.

    Edges are partitioned by (dst-owner core, dst-tile group, src PIECE)
    where a piece is a contiguous range of local-node tiles small enough
    that C*piece_rows fits in int16 gather indices."""
    N, NPC, NL, G = cfg["N"], cfg["NPC"], cfg["NL"], cfg["G"]
    PIECES = cfg["PIECES"]
    NP = len(PIECES)
    TILES = NL // 128
    assert sum(PIECES) == TILES
    pstart = np.cumsum([0] + PIECES)          # tile starts per piece
    NGRP = (TILES + G - 1) // G

    src = np.asarray(edge_index[0], dtype=np.int64)
    dst = np.asarray(edge_index[1], dtype=np.int64)
    deg = np.bincount(dst, minlength=N).astype(np.float32) + 1.0

    owner = dst // NPC
    dst_loc = (dst - owner * NPC).astype(np.int64)
    sowner = src // NPC
    sloc = (src - sowner * NPC).astype(np.int64)
    stile = sloc // 128
    piece = np.searchsorted(pstart[1:-1], stile, side="right")
    prow = sowner * (np.array(PIECES)[piece] * 128) + (sloc - pstart[piece] * 128)
    assert prow.max() < 32768
    dtile = dst_loc // 128
    grp = dtile // G

    # per (core, grp, piece): edge lists sorted by dst
    seg_edges = {}
    for c in range(C):
        m_c = owner == c
        for g in range(NGRP):
            m_g = m_c & (grp == g)
            for p in range(NP):
                m = m_g & (piece == p)
                idx = np.nonzero(m)[0]
                order = np.argsort(dst_loc[idx], kind="stable")
                seg_edges[(c, g, p)] = idx[order]

    # common segment sizes (max over cores, padded to 128)
    seg_size = {}
    for g in range(NGRP):
        for p in range(NP):
            mx = max(len(seg_edges[(c, g, p)]) for c in range(C))
            seg_size[(g, p)] = ((mx + 127) // 128) * 128

    # batches + ops (common structure); CD order: group-major, piece inner
    batches = []   # (grp, piece, size, seg_off)
    ops = []       # (batch_id, col, gtile)
    for g in range(NGRP):
        for p in range(NP):
            S = seg_size[(g, p)]
            if S == 0:
                continue
            nsub = S // 128
            lo = np.full(nsub, 10 ** 9, dtype=np.int64)
            hi = np.full(nsub, -1, dtype=np.int64)
            for c in range(C):
                idx = seg_edges[(c, g, p)]
                if len(idx) == 0:
                    continue
                dt = dtile[idx]
                for s in range((len(idx) + 127) // 128):
                    a, b = s * 128, min((s + 1) * 128, len(idx))
                    lo[s] = min(lo[s], dt[a:b].min())
                    hi[s] = max(hi[s], dt[a:b].max())
            off = 0
            while off < S:
                size = min(NI_MAX, S - off)
                b_id = len(batches)
                batches.append((g, p, size, off))
                for col in range(size // 128):
                    s = (off // 128) + col
                    if hi[s] < 0:
                        lo[s] = hi[s] = g * G
                    for t in range(int(lo[s]), int(hi[s]) + 1):
                        ops.append((b_id, col, t))
                off += size

    # guarantee every tile gets at least one op (so finish_tile fires)
    covered = set(t for (_, _, t) in ops)
    for t in range(TILES):
        if t not in covered:
            # append a no-edge op on the tile's group's last batch
            g = t // G
            bsel = max(b_id for b_id, (gg, _, _, _) in enumerate(batches)
                       if gg == g)
            ops.append((bsel, 0, t))
    # re-sort ops so each batch's ops are contiguous in batch order
    ops.sort(key=lambda o: (o[0], o[1]))

    NBAT, NOPS = len(batches), len(ops)
    first_op, last_op = {}, {}
    for i, (_, _, t) in enumerate(ops):
        if t not in first_op:
            first_op[t] = i
        last_op[t] = i

    def wrap16_rep(vals, ncols):
        a = np.zeros(16 * ncols, dtype=np.int16)
        a[: len(vals)] = vals
        w = a.reshape(ncols, 16).T
        return np.tile(w, (8, 1))  # (128, ncols)

    col_off = []
    acc_cols = 0
    for (g, p, size, off) in batches:
        col_off.append(acc_cols)
        acc_cols += size // 16
    gidx_data = np.zeros((C, 128, acc_cols), dtype=np.int16)
    dloc_data = np.full((C, 128, NOPS), -1000.0, dtype=np.float32)
    for c in range(C):
        for b_id, (g, p, size, off) in enumerate(batches):
            idx = seg_edges[(c, g, p)]
            pos = idx[off: off + size]
            gi = np.zeros(size, dtype=np.int16)
            gi[: len(pos)] = prow[pos].astype(np.int16)
            co = col_off[b_id]
            gidx_data[c, :, co: co + size // 16] = wrap16_rep(gi, size // 16)
    for c in range(C):
        for o_id, (b_id, col, t) in enumerate(ops):
            g, p, size, off = batches[b_id]
            idx = seg_edges[(c, g, p)]
            a = off + col * 128
            pos = idx[a: a + 128]
            if len(pos):
                v = dst_loc[pos].astype(np.float32) - t * 128.0
                dloc_data[c, : len(pos), o_id] = v

    return dict(
        deg=deg, batches=batches, ops=ops, first_op=first_op, last_op=last_op,
        gidx_data=gidx_data, dloc_data=dloc_data, col_off=col_off,
        GIDX_COLS=acc_cols, pstart=pstart,
        TILES=TILES, NGRP=NGRP, NP=NP, NBAT=NBAT, NOPS=NOPS,
    )


def build_graph(cfg, prep, params, num_msg_bufs=26):
    """Build the SPMD Bass graph. params: numpy dict (W1,W2,W3,b3,g1,be1,g2,be2)."""
    import sys
    sys.path.insert(0, "/opt/trn_rl_repo")
    from concourse import bacc, tile
    import concourse.mybir as mybir

    N, NPC, NL, G, IN = (cfg[k] for k in ["N", "NPC", "NL", "G", "IN"])
    PIECES = cfg["PIECES"]
    TILES, NGRP, NP = prep["TILES"], prep["NGRP"], prep["NP"]
    NBAT, NOPS = prep["NBAT"], prep["NOPS"]
    batches, ops = prep["batches"], prep["ops"]
    col_off, GIDX_COLS = prep["col_off"], prep["GIDX_COLS"]
    first_op, last_op = prep["first_op"], prep["last_op"]
    pstart = prep["pstart"]
    f32, bf16, i16 = mybir.dt.float32, mybir.dt.bfloat16, mybir.dt.int16

    nc = bacc.Bacc("TRN2", target_bir_lowering=False, num_swdge_queues=4)

    # ---- DRAM I/O ----
    xT_d = nc.dram_tensor("xT", [IN, NL], bf16, kind="ExternalInput")
    degc_d = nc.dram_tensor("degc", [128, TILES], f32, kind="ExternalInput")
    maskc_d = nc.dram_tensor("maskc", [128, TILES], bf16, kind="ExternalInput")
    gidx_d = nc.dram_tensor("gidx", [128, GIDX_COLS], i16, kind="ExternalInput")
    dloc_d = nc.dram_tensor("dloc", [128, NOPS], f32, kind="ExternalInput")
    W1_d = nc.dram_tensor("W1", [IN, H], bf16, kind="ExternalInput")
    W2_d = nc.dram_tensor("W2", [H, H], bf16, kind="ExternalInput")
    W3_d = nc.dram_tensor("W3", [H, 1], bf16, kind="ExternalInput")
    gbe_d = nc.dram_tensor("gbe", [128, 4], f32, kind="ExternalInput")  # g1,be1,g2,be2
    iden_d = nc.dram_tensor("iden", [128, 128], bf16, kind="ExternalInput")
    iota_d = nc.dram_tensor("iota", [128, 128], bf16, kind="ExternalInput")
    onesr_d = nc.dram_tensor("onesr", [1, 128], bf16, kind="ExternalInput")
    out_d = nc.dram_tensor("out", [128, TILES], f32, kind="ExternalOutput")

    # per (layer, piece) table shards + gathered tables
    tbl_loc = [[nc.dram_tensor(f"tbl{L}p{p}_loc", [PIECES[p] * 128, H], bf16)
                for p in range(NP)] for L in range(3)]
    tbl_full = [[nc.dram_tensor(f"tbl{L}p{p}_full", [C * PIECES[p] * 128, H],
                                bf16, addr_space="Shared")
                 for p in range(NP)] for L in range(3)]
    st_in = [nc.dram_tensor(f"st{L}_in", [128, 2], f32) for L in range(2)]
    st_out = [nc.dram_tensor(f"st{L}_out", [128, 2], f32, addr_space="Shared")
              for L in range(2)]
    prime_in = nc.dram_tensor("prime_in", [1, 16], f32)
    prime_out = nc.dram_tensor("prime_out", [C, 16], f32, addr_space="Shared")

    b3 = float(params["b3"][0])
    rg = [list(range(C))]

    # 512-col projection chunks, aligned to piece boundaries
    chunks = []   # (piece, col_a, col_b)  [cols in node axis]
    for p in range(NP):
        a, b = pstart[p] * 128, pstart[p + 1] * 128
        j = a
        while j < b:
            w = min(512, b - j)
            chunks.append((p, j, j + w))
            j += w
    NSL = len(chunks)

    from contextlib import ExitStack
    with tile.TileContext(nc) as tc, ExitStack() as ctx:
        res = ctx.enter_context(tc.tile_pool(name="res", bufs=1))
        mtp = ctx.enter_context(tc.tile_pool(name="mtp", bufs=3))
        stg = ctx.enter_context(tc.tile_pool(name="stg", bufs=4))
        msg = ctx.enter_context(tc.tile_pool(name="msg", bufs=num_msg_bufs))
        ohp = ctx.enter_context(tc.tile_pool(name="ohp", bufs=6))
        hpp = ctx.enter_context(tc.tile_pool(name="hpp", bufs=6))
        sqp = ctx.enter_context(tc.tile_pool(name="sqp", bufs=4))
        colp = ctx.enter_context(tc.tile_pool(name="colp", bufs=4))
        psA = ctx.enter_context(tc.tile_pool(name="psA", bufs=1, space="PSUM"))
        psB = ctx.enter_context(tc.tile_pool(name="psB", bufs=2, space="PSUM"))
        psACC = ctx.enter_context(tc.tile_pool(name="psACC", bufs=1, space="PSUM"))
        psST = ctx.enter_context(tc.tile_pool(name="psST", bufs=1, space="PSUM"))
        if True:
            # ---- prime the collective stream (absorbs barrier + first
            # trigger latency off the critical path) ----
            nc.gpsimd.collective_compute(
                "AllGather", mybir.AluOpType.bypass,
                ins=[prime_in[:]], outs=[prime_out[:]], replica_groups=rg)

            # ---- residents ----
            gidx = res.tile([128, GIDX_COLS], i16, tag="gidx")
            nc.sync.dma_start(out=gidx[:], in_=gidx_d[:, :])
            dloc = res.tile([128, NOPS], f32, tag="dloc")
            nc.sync.dma_start(out=dloc[:], in_=dloc_d[:, :])
            W1 = res.tile([IN, H], bf16, tag="W1")
            nc.sync.dma_start(out=W1[:], in_=W1_d[:, :])
            W2 = res.tile([H, H], bf16, tag="W2")
            nc.sync.dma_start(out=W2[:], in_=W2_d[:, :])
            W3 = res.tile([H, 1], bf16, tag="W3")
            nc.sync.dma_start(out=W3[:], in_=W3_d[:, :])
            gbe = res.tile([128, 4], f32, tag="gbe")
            nc.sync.dma_start(out=gbe[:], in_=gbe_d[:, :])
            iden = res.tile([128, 128], bf16, tag="iden")
            nc.sync.dma_start(out=iden[:], in_=iden_d[:, :])
            iota = res.tile([128, 128], bf16, tag="iota")
            nc.sync.dma_start(out=iota[:], in_=iota_d[:, :])
            onesr = res.tile([1, 128], bf16, tag="onesr")
            nc.sync.dma_start(out=onesr[:], in_=onesr_d[:, :])
            maskc = res.tile([128, TILES], bf16, tag="maskc")
            nc.sync.dma_start(out=maskc[:], in_=maskc_d[:, :])
            degc = res.tile([128, TILES], f32, tag="degc")
            nc.sync.dma_start(out=degc[:], in_=degc_d[:, :])

            # dis (node-major cols): dis = sqrt(1/deg)
            disc = res.tile([128, TILES], f32, tag="disc")
            nc.vector.reciprocal(out=disc[:], in_=degc[:])
            nc.scalar.sqrt(out=disc[:], in_=disc[:])

            def mkchunks(prefix):
                out = []
                for j, (p, a, b) in enumerate(chunks):
                    out.append(res.tile([128, b - a], bf16, tag=f"{prefix}{j}",
                                        name=f"{prefix}{j}"))
                return out
            hta = mkchunks("hta")
            htb = mkchunks("htb")
            hpre_t = [res.tile([128, 128], bf16, tag=f"hpre{t}", name=f"hpre{t}")
                      for t in range(TILES)]
            o_sb = res.tile([128, TILES], f32, tag="o_sb")

            def piece_of_tile(t):
                for p in range(NP):
                    if t < pstart[p + 1]:
                        return p
                raise AssertionError

            def phase_A(L, hin, Wt, AB):
                """[BN-apply chunk] + project + transpose + dis-scale ->
                per-piece table shard -> per-piece AllGather."""
                for j, (p, a, b) in enumerate(chunks):
                    if L > 0 and AB is not None:
                        # fused BN affine + relu from hpre tiles (feat-major)
                        for t in range(a // 128, b // 128):
                            r = t - a // 128
                            nc.scalar.activation(
                                hin[j][:, r * 128:(r + 1) * 128], hpre_t[t][:],
                                mybir.ActivationFunctionType.Relu,
                                scale=AB[:, 0:1], bias=AB[:, 1:2])
                    if L < 2:
                        if L == 0:
                            xc = mtp.tile([IN, 512], bf16, tag="xc")
                            nc.sync.dma_start(out=xc[:, : b - a], in_=xT_d[:, a:b])
                            rhs = xc[:, : b - a]
                        else:
                            rhs = hin[j][:, : b - a]
                        pa = psA.tile([128, 512], f32, tag="psA")
                        nc.tensor.matmul(pa[:, : b - a], Wt[:], rhs,
                                         start=True, stop=True)
                        mt = mtp.tile([128, 512], bf16, tag="mt")
                        nc.scalar.activation(mt[:, : b - a], pa[:, : b - a],
                                             mybir.ActivationFunctionType.Copy)
                        for jj in range((b - a) // 128):
                            t = (a // 128) + jj
                            tt = t - pstart[p]
                            pb = psB.tile([128, 128], f32, tag="psB")
                            nc.tensor.matmul(pb[:], mt[:, jj * 128:(jj + 1) * 128],
                                             iden[:], start=True, stop=True)
                            sg = stg.tile([128, 128], bf16, tag="stg")
                            nc.vector.tensor_scalar_mul(
                                out=sg[:], in0=pb[:], scalar1=disc[:, t: t + 1])
                            nc.sync.dma_start(
                                out=tbl_loc[L][p][tt * 128:(tt + 1) * 128, :],
                                in_=sg[:])
                    else:
                        # L3: m3 = W3^T @ h2T; replicate row to node-major tiles
                        pa = psA.tile([1, 512], f32, tag="psA")
                        nc.tensor.matmul(pa[:, : b - a], Wt[:], hin[j][:, : b - a],
                                         start=True, stop=True)
                        m3c = mtp.tile([1, 512], bf16, tag="m3c")
                        nc.scalar.activation(m3c[:, : b - a], pa[:, : b - a],
                                             mybir.ActivationFunctionType.Copy)
                        for jj in range((b - a) // 128):
                            t = (a // 128) + jj
                            tt = t - pstart[p]
                            pb = psB.tile([128, 128], f32, tag="psB")
                            nc.tensor.matmul(pb[:], m3c[:, jj * 128:(jj + 1) * 128],
                                             onesr[:], start=True, stop=True)
                            sg = stg.tile([128, 128], bf16, tag="stg")
                            nc.vector.tensor_scalar_mul(
                                out=sg[:], in0=pb[:], scalar1=disc[:, t: t + 1])
                            nc.sync.dma_start(
                                out=tbl_loc[L][p][tt * 128:(tt + 1) * 128, :],
                                in_=sg[:])
                    # last chunk of this piece -> fire its AllGather
                    if j + 1 == NSL or chunks[j + 1][0] != p:
                        nc.gpsimd.collective_compute(
                            "AllGather", mybir.AluOpType.bypass,
                            ins=[tbl_loc[L][p][:]], outs=[tbl_full[L][p][:]],
                            replica_groups=rg)

            def phase_CD(L):
                FW = 1 if L == 2 else H     # scatter rhs feature width
                sts = None
                if L < 2:
                    sts = psST.tile([128, 2], f32, tag="sts")
                    nc.vector.memset(sts[:], 0.0)
                ops_span = {}
                for o_id, (bb, _, _) in enumerate(ops):
                    if bb not in ops_span:
                        ops_span[bb] = [o_id, 0]
                    ops_span[bb][1] += 1
                cur_g = -1
                pst = None
                for bb in range(NBAT):
                    g, p, size, off = batches[bb]
                    if g != cur_g:
                        cur_g = g
                        pst = psACC.tile([128, G, FW], f32, tag="acc",
                                         name=f"accL{L}g{g}")
                        nc.vector.memset(pst[:], 0.0)
                    m = msg.tile([128, NI_MAX // 128, H], bf16, tag="msg")
                    co = col_off[bb]
                    rows = C * PIECES[p] * 128
                    nc.gpsimd.dma_gather(
                        out_ap=m[:, : size // 128, :],
                        in_ap=tbl_full[L][p][0: rows, :],
                        idxs_ap=gidx[:, co: co + size // 16],
                        num_idxs=size, num_idxs_reg=size, elem_size=H,
                        queue_num=bb % 4)
                    o0, cnt = ops_span[bb]
                    oh = ohp.tile([128, cnt, 128], bf16, tag="oh",
                                  name=f"ohL{L}b{bb}")
                    for j in range(cnt):
                        nc.vector.tensor_scalar(
                            oh[:, j, :], iota[:], dloc[:, o0 + j: o0 + j + 1],
                            None, mybir.AluOpType.is_equal)
                    for j in range(cnt):
                        o_id = o0 + j
                        _, col, t = ops[o_id]
                        ti = t - g * G
                        nc.tensor.matmul(pst[:, ti, :], oh[:, j, :],
                                         m[:, col, 0:FW],
                                         start=False, stop=False,
                                         skip_group_check=True)
                        if last_op[t] == o_id:
                            finish_tile(L, t, pst, ti, sts)
                if L < 2:
                    return finish_layer_stats(L, sts)
                nc.sync.dma_start(out=out_d[:, :], in_=o_sb[:])
                return None

            def finish_tile(L, t, pst, ti, sts):
                p = piece_of_tile(t)
                tt = t - pstart[p]
                if L == 2:
                    # self-loop: add table scalar (col 0 of replicated tile)
                    tbt = stg.tile([128, 1], bf16, tag="tbt1", name=f"tbtL2t{t}")
                    nc.sync.dma_start(
                        out=tbt[:], in_=tbl_loc[L][p][tt * 128:(tt + 1) * 128, 0:1])
                    nc.tensor.matmul(pst[:, ti, :], iden[:], tbt[:],
                                     start=False, stop=False, skip_group_check=True)
                    nc.scalar.activation(o_sb[:, t: t + 1], pst[:, ti, 0:1],
                                         mybir.ActivationFunctionType.Sigmoid,
                                         scale=disc[:, t: t + 1], bias=b3)
                    return
                # self-loop term: acc_t += I @ tbl_tile_t  (tbl = dis*m)
                tbt = stg.tile([128, 128], bf16, tag="tbt", name=f"tbtL{L}t{t}")
                nc.sync.dma_start(out=tbt[:],
                                  in_=tbl_loc[L][p][tt * 128:(tt + 1) * 128, :])
                nc.tensor.matmul(pst[:, ti, :], iden[:], tbt[:],
                                 start=False, stop=False, skip_group_check=True)
                hp = hpp.tile([128, 128], bf16, tag="hp")
                nc.scalar.activation(hp[:], pst[:, ti, :],
                                     mybir.ActivationFunctionType.Copy,
                                     scale=disc[:, t: t + 1])
                sq = sqp.tile([128, 128], bf16, tag="sq")
                nc.scalar.activation(sq[:], hp[:],
                                     mybir.ActivationFunctionType.Square)
                nc.tensor.matmul(sts[:, 0:1], hp[:], maskc[:, t: t + 1],
                                 start=False, stop=False, skip_group_check=True)
                nc.tensor.matmul(sts[:, 1:2], sq[:], maskc[:, t: t + 1],
                                 start=False, stop=False, skip_group_check=True)
                pb = psB.tile([128, 128], f32, tag="psB")
                nc.tensor.matmul(pb[:], hp[:], iden[:], start=True, stop=True)
                nc.scalar.activation(hpre_t[t][:], pb[:],
                                     mybir.ActivationFunctionType.Copy)

            def finish_layer_stats(L, sts):
                """stats AllReduce -> BN affine coefficients (apply is fused
                into the next phase_A)."""
                stat = colp.tile([128, 2], f32, tag="stat")
                nc.vector.tensor_copy(out=stat[:], in_=sts[:])
                nc.sync.dma_start(out=st_in[L][:, :], in_=stat[:])
                nc.gpsimd.collective_compute(
                    "AllReduce", mybir.AluOpType.add,
                    ins=[st_in[L][:]], outs=[st_out[L][:]], replica_groups=rg)
                stg_ = colp.tile([128, 2], f32, tag="statg")
                nc.sync.dma_start(out=stg_[:], in_=st_out[L][:, :])
                mu = colp.tile([128, 4], f32, tag="mu")
                inv_n = 1.0 / float(N)
                nc.vector.tensor_scalar_mul(out=mu[:, 0:2], in0=stg_[:], scalar1=inv_n)
                nc.vector.tensor_tensor(out=mu[:, 2:3], in0=mu[:, 0:1], in1=mu[:, 0:1],
                                        op=mybir.AluOpType.mult)
                nc.vector.tensor_tensor(out=mu[:, 2:3], in0=mu[:, 1:2], in1=mu[:, 2:3],
                                        op=mybir.AluOpType.subtract)
                nc.vector.tensor_scalar_add(out=mu[:, 2:3], in0=mu[:, 2:3],
                                            scalar1=BN_EPS)
                nc.vector.reciprocal(out=mu[:, 3:4], in_=mu[:, 2:3])
                nc.scalar.sqrt(out=mu[:, 3:4], in_=mu[:, 3:4])
                AB = colp.tile([128, 2], f32, tag=f"AB{L}", name=f"AB{L}")
                gcol = gbe[:, 2 * L: 2 * L + 1]
                becol = gbe[:, 2 * L + 1: 2 * L + 2]
                nc.vector.tensor_tensor(out=AB[:, 0:1], in0=gcol, in1=mu[:, 3:4],
                                        op=mybir.AluOpType.mult)
                nc.vector.tensor_tensor(out=AB[:, 1:2], in0=mu[:, 0:1], in1=AB[:, 0:1],
                                        op=mybir.AluOpType.mult)
                nc.vector.tensor_tensor(out=AB[:, 1:2], in0=becol, in1=AB[:, 1:2],
                                        op=mybir.AluOpType.subtract)
                return AB

            # ---- run 3 layers ----
            ab_of = {}
            for L in range(3):
                hin = None if L == 0 else (htb if L == 1 else hta)
                Wt = [W1, W2, W3][L]
                phase_A(L, hin, Wt, ab_of.get(L - 1))
                ab_of[L] = phase_CD(L)

    nc.finalize()
    return nc


def make_inputs(cfg, prep, inputs, core):
    """Per-core input map."""
    N, NPC, NL, IN = cfg["N"], cfg["NPC"], cfg["NL"], cfg["IN"]
    TILES = NL // 128
    bf = ml_dtypes.bfloat16
    x = np.asarray(inputs["x"], np.float32)
    deg = prep["deg"]

    xl = np.zeros((NL, IN), np.float32)
    xl[:NPC] = x[core * NPC:(core + 1) * NPC]
    degl = np.ones(NL, np.float32)
    degl[:NPC] = deg[core * NPC:(core + 1) * NPC]
    mask = np.zeros(NL, np.float32)
    mask[:NPC] = 1.0

    gbe = np.stack([
        np.asarray(inputs["g1"], np.float32), np.asarray(inputs["be1"], np.float32),
        np.asarray(inputs["g2"], np.float32), np.asarray(inputs["be2"], np.float32),
    ], axis=1)  # (128, 4)

    return {
        "xT": xl.T.astype(bf).copy(),
        "degc": degl.reshape(TILES, 128).T.copy(),
        "maskc": mask.reshape(TILES, 128).T.astype(bf).copy(),
        "gidx": prep["gidx_data"][core],
        "dloc": prep["dloc_data"][core],
        "W1": np.asarray(inputs["W1"], np.float32).astype(bf),
        "W2": np.asarray(inputs["W2"], np.float32).astype(bf),
        "W3": np.asarray(inputs["W3"], np.float32).astype(bf),
        "gbe": gbe,
        "iden": np.eye(128, dtype=np.float32).astype(bf),
        "iota": np.tile(np.arange(128, dtype=np.float32), (128, 1)).astype(bf),
        "onesr": np.ones((1, 128), np.float32).astype(bf),
    }


def unshard_output(cfg, results):
    N, NPC, NL = cfg["N"], cfg["NPC"], cfg["NL"]
    TILES = NL // 128
    out = np.zeros((N, 1), np.float32)
    for c in range(C):
        o = results[c]["out"]            # (128, TILES)
        flat = o.T.reshape(NL)           # node-major
        out[c * NPC:(c + 1) * NPC, 0] = flat[:NPC]
    return out


def _ensure_axon_hooks_shim():
    """bass_utils' trace path imports antenv.axon_hooks, which this image
    lacks; register a no-op so a stray BASS_TRACE=1 can't crash the run."""
    import types
    if 'antenv.axon_hooks' in sys.modules:
        return
    try:
        import antenv
        from antenv import axon_hooks  # noqa: F401
    except ImportError:
        mod = types.ModuleType('antenv.axon_hooks')
        _hook = [None]
        mod.set_axon_ntff_profile_hook = lambda h: _hook.__setitem__(0, h)
        mod.get_axon_ntff_profile_hook = lambda: _hook[0]
        sys.modules['antenv.axon_hooks'] = mod
        try:
            antenv.axon_hooks = mod
        except Exception:
            pass


def kernel(**inputs):
    import os
    import numpy as np
    from concourse import bass_utils

    _ensure_axon_hooks_shim()
    cfg = make_cfg(full=True)
    inputs = {k: np.asarray(v) for k, v in inputs.items()}
    prep = host_prep(cfg, inputs["edge_index"])
    nc = build_graph(cfg, prep, inputs)
    in_maps = [make_inputs(cfg, prep, inputs, c) for c in range(C)]
    prev = os.environ.get("BASS_NEVER_TRACE")
    os.environ["BASS_NEVER_TRACE"] = "1"
    try:
        res = bass_utils.run_bass_kernel_spmd(nc, in_maps, list(range(C)), trace=False)
    finally:
        if prev is None:
            os.environ.pop("BASS_NEVER_TRACE", None)
        else:
            os.environ["BASS_NEVER_TRACE"] = prev
    return unshard_output(cfg, [res.results[c] for c in range(C)])


# revision 8
# speedup vs baseline: 1.4144x; 1.4144x over previous
"""Self-contained Trainium2 Bass kernel for the 3-layer GCN (AgriGraphGCN).

kernel(**inputs) -> (100000, 1) float32 risk scores, computed SPMD on 8
NeuronCores. Strategy: shard nodes by dst owner; per layer build a bf16
feature table (project + PE-transpose + degree scaling) in 2 PIECES, each
piece AllGather'd as soon as written so cross-core gathers overlap the
second collective; dma_gather per-edge source rows (4 SWDGE queues),
scatter via one-hot PE matmuls into PSUM accumulators (single column for
L3's scalar messages), BatchNorm from AllReduce'd masked stats with the
affine+relu application fused into the next layer's projection pass.
"""
import sys
sys.path.insert(0, "/opt/trn_rl_repo")

import numpy as np
import ml_dtypes


def make_cfg(full=True):
    if full:
        return dict(N=100000, E=640000, NPC=12500, NL=12544, G=14, IN=6,
                    PIECES=[49, 49])
    return dict(N=4096, E=16384, NPC=512, NL=512, G=2, IN=6, PIECES=[2, 2])


C = 8           # cores
H = 128         # hidden
NI_MAX = 1024   # dma_gather per-call limit
HWIN = 32768    # int16 index window (rows per gather source view)
BN_EPS = 1e-5


def host_prep(cfg, edge_index):
    """Build common (cross-core) batch/op structure + per-core index data.

    Edges are partitioned by (dst-owner core, dst-tile group, src PIECE,
    idx-half) where a piece is a contiguous range of local-node tiles
    AllGather'd as one unit, and the half splits the gathered piece into
    int16-addressable windows."""
    N, NPC, NL, G = cfg["N"], cfg["NPC"], cfg["NL"], cfg["G"]
    PIECES = cfg["PIECES"]
    NP = len(PIECES)
    TILES = NL // 128
    assert sum(PIECES) == TILES
    pstart = np.cumsum([0] + PIECES)          # tile starts per piece
    NGRP = (TILES + G - 1) // G
    NH = [max(1, (C * PIECES[p] * 128 + HWIN - 1) // HWIN) for p in range(NP)]

    src = np.asarray(edge_index[0], dtype=np.int64)
    dst = np.asarray(edge_index[1], dtype=np.int64)
    deg = np.bincount(dst, minlength=N).astype(np.float32) + 1.0

    owner = dst // NPC
    dst_loc = (dst - owner * NPC).astype(np.int64)
    sowner = src // NPC
    sloc = (src - sowner * NPC).astype(np.int64)
    stile = sloc // 128
    piece = np.searchsorted(pstart[1:-1], stile, side="right")
    prow = sowner * (np.array(PIECES)[piece] * 128) + (sloc - pstart[piece] * 128)
    half = prow // HWIN
    hidx = prow - half * HWIN
    assert hidx.max() < HWIN
    dtile = dst_loc // 128
    grp = dtile // G

    # cells in CD emission order: (g, p, h)
    cells = [(g, p, h) for g in range(NGRP) for p in range(NP)
             for h in range(NH[p])]

    seg_edges = {}
    for c in range(C):
        m_c = owner == c
        for (g, p, h) in cells:
            m = m_c & (grp == g) & (piece == p) & (half == h)
            idx = np.nonzero(m)[0]
            order = np.argsort(dst_loc[idx], kind="stable")
            seg_edges[(c, g, p, h)] = idx[order]

    seg_size = {}
    for cell in cells:
        mx = max(len(seg_edges[(c,) + cell]) for c in range(C))
        seg_size[cell] = ((mx + 127) // 128) * 128

    batches = []   # (g, p, h, size, seg_off)
    ops = []       # (batch_id, col, gtile)
    for cell in cells:
        g, p, h = cell
        S = seg_size[cell]
        if S == 0:
            continue
        nsub = S // 128
        lo = np.full(nsub, 10 ** 9, dtype=np.int64)
        hi = np.full(nsub, -1, dtype=np.int64)
        for c in range(C):
            idx = seg_edges[(c,) + cell]
            if len(idx) == 0:
                continue
            dt = dtile[idx]
            for s in range((len(idx) + 127) // 128):
                a, b = s * 128, min((s + 1) * 128, len(idx))
                lo[s] = min(lo[s], dt[a:b].min())
                hi[s] = max(hi[s], dt[a:b].max())
        off = 0
        while off < S:
            size = min(NI_MAX, S - off)
            b_id = len(batches)
            batches.append((g, p, h, size, off))
            for col in range(size // 128):
                s = (off // 128) + col
                if hi[s] < 0:
                    lo[s] = hi[s] = g * G
                for t in range(int(lo[s]), int(hi[s]) + 1):
                    ops.append((b_id, col, t))
            off += size

    # guarantee every tile gets at least one op (so finish_tile fires)
    covered = set(t for (_, _, t) in ops)
    for t in range(TILES):
        if t not in covered:
            g = t // G
            bsel = max(b_id for b_id, bt in enumerate(batches) if bt[0] == g)
            ops.append((bsel, 0, t))
    ops.sort(key=lambda o: (o[0], o[1]))

    NBAT, NOPS = len(batches), len(ops)
    first_op, last_op = {}, {}
    for i, (_, _, t) in enumerate(ops):
        if t not in first_op:
            first_op[t] = i
        last_op[t] = i

    def wrap16_rep(vals, ncols):
        a = np.zeros(16 * ncols, dtype=np.int16)
        a[: len(vals)] = vals
        w = a.reshape(ncols, 16).T
        return np.tile(w, (8, 1))  # (128, ncols)

    col_off = []
    acc_cols = 0
    for (g, p, h, size, off) in batches:
        col_off.append(acc_cols)
        acc_cols += size // 16
    gidx_data = np.zeros((C, 128, acc_cols), dtype=np.int16)
    dloc_data = np.full((C, 128, NOPS), -1000.0, dtype=np.float32)
    for c in range(C):
        for b_id, (g, p, h, size, off) in enumerate(batches):
            idx = seg_edges[(c, g, p, h)]
            pos = idx[off: off + size]
            gi = np.zeros(size, dtype=np.int16)
            gi[: len(pos)] = hidx[pos].astype(np.int16)
            co = col_off[b_id]
            gidx_data[c, :, co: co + size // 16] = wrap16_rep(gi, size // 16)
    for c in range(C):
        for o_id, (b_id, col, t) in enumerate(ops):
            g, p, h, size, off = batches[b_id]
            idx = seg_edges[(c, g, p, h)]
            a = off + col * 128
            pos = idx[a: a + 128]
            if len(pos):
                v = dst_loc[pos].astype(np.float32) - t * 128.0
                dloc_data[c, : len(pos), o_id] = v

    return dict(
        deg=deg, batches=batches, ops=ops, first_op=first_op, last_op=last_op,
        gidx_data=gidx_data, dloc_data=dloc_data, col_off=col_off,
        GIDX_COLS=acc_cols, pstart=pstart, NH=NH,
        TILES=TILES, NGRP=NGRP, NP=NP, NBAT=NBAT, NOPS=NOPS,
    )


def build_graph(cfg, prep, params, num_msg_bufs=22):
    """Build the SPMD Bass graph. params: numpy dict (W1,W2,W3,b3,g1,be1,g2,be2)."""
    import sys
    sys.path.insert(0, "/opt/trn_rl_repo")
    from concourse import bacc, tile
    import concourse.mybir as mybir

    N, NPC, NL, G, IN = (cfg[k] for k in ["N", "NPC", "NL", "G", "IN"])
    PIECES = cfg["PIECES"]
    TILES, NGRP, NP = prep["TILES"], prep["NGRP"], prep["NP"]
    NBAT, NOPS = prep["NBAT"], prep["NOPS"]
    batches, ops = prep["batches"], prep["ops"]
    col_off, GIDX_COLS = prep["col_off"], prep["GIDX_COLS"]
    last_op = prep["last_op"]
    pstart = prep["pstart"]
    f32, bf16, i16 = mybir.dt.float32, mybir.dt.bfloat16, mybir.dt.int16

    nc = bacc.Bacc("TRN2", target_bir_lowering=False, num_swdge_queues=4)

    # ---- DRAM I/O ----
    xT_d = nc.dram_tensor("xT", [IN, NL], bf16, kind="ExternalInput")
    degc_d = nc.dram_tensor("degc", [128, TILES], f32, kind="ExternalInput")
    maskc_d = nc.dram_tensor("maskc", [128, TILES], bf16, kind="ExternalInput")
    gidx_d = nc.dram_tensor("gidx", [128, GIDX_COLS], i16, kind="ExternalInput")
    dloc_d = nc.dram_tensor("dloc", [128, NOPS], bf16, kind="ExternalInput")
    W1_d = nc.dram_tensor("W1", [IN, H], bf16, kind="ExternalInput")
    W2_d = nc.dram_tensor("W2", [H, H], bf16, kind="ExternalInput")
    W3_d = nc.dram_tensor("W3", [H, 1], bf16, kind="ExternalInput")
    gbe_d = nc.dram_tensor("gbe", [128, 4], f32, kind="ExternalInput")  # g1,be1,g2,be2
    iden_d = nc.dram_tensor("iden", [128, 128], bf16, kind="ExternalInput")
    iota_d = nc.dram_tensor("iota", [128, 128], bf16, kind="ExternalInput")
    onesr_d = nc.dram_tensor("onesr", [1, 128], bf16, kind="ExternalInput")
    out_d = nc.dram_tensor("out", [128, TILES], f32, kind="ExternalOutput")

    tbl_loc = [[nc.dram_tensor(f"tbl{L}p{p}_loc", [PIECES[p] * 128, H], bf16)
                for p in range(NP)] for L in range(3)]
    tbl_full = [[nc.dram_tensor(f"tbl{L}p{p}_full", [C * PIECES[p] * 128, H],
                                bf16, addr_space="Shared")
                 for p in range(NP)] for L in range(3)]
    st_in = [nc.dram_tensor(f"st{L}_in", [128, 2], f32) for L in range(2)]
    st_out = [nc.dram_tensor(f"st{L}_out", [128, 2], f32, addr_space="Shared")
              for L in range(2)]
    prime_in = nc.dram_tensor("prime_in", [1, 16], f32)
    prime_out = nc.dram_tensor("prime_out", [C, 16], f32, addr_space="Shared")

    b3 = float(params["b3"][0])
    rg = [list(range(C))]

    # 512-col projection chunks, aligned to piece boundaries
    chunks = []   # (piece, col_a, col_b)
    for p in range(NP):
        a, b = pstart[p] * 128, pstart[p + 1] * 128
        j = a
        while j < b:
            w = min(512, b - j)
            chunks.append((p, j, j + w))
            j += w
    NSL = len(chunks)

    from contextlib import ExitStack
    with tile.TileContext(nc) as tc, ExitStack() as ctx:
        res = ctx.enter_context(tc.tile_pool(name="res", bufs=1))
        mtp = ctx.enter_context(tc.tile_pool(name="mtp", bufs=3))
        stg = ctx.enter_context(tc.tile_pool(name="stg", bufs=3))
        tbp = ctx.enter_context(tc.tile_pool(name="tbp", bufs=4))
        msg = ctx.enter_context(tc.tile_pool(name="msg", bufs=num_msg_bufs))
        ohp = ctx.enter_context(tc.tile_pool(name="ohp", bufs=6))
        hpp = ctx.enter_context(tc.tile_pool(name="hpp", bufs=6))
        sqp = ctx.enter_context(tc.tile_pool(name="sqp", bufs=4))
        colp = ctx.enter_context(tc.tile_pool(name="colp", bufs=4))
        psA = ctx.enter_context(tc.tile_pool(name="psA", bufs=1, space="PSUM"))
        psB = ctx.enter_context(tc.tile_pool(name="psB", bufs=2, space="PSUM"))
        psACC = ctx.enter_context(tc.tile_pool(name="psACC", bufs=1, space="PSUM"))
        psST = ctx.enter_context(tc.tile_pool(name="psST", bufs=1, space="PSUM"))
        if True:
            # ---- prime the collective stream ----
            nc.gpsimd.collective_compute(
                "AllGather", mybir.AluOpType.bypass,
                ins=[prime_in[:]], outs=[prime_out[:]], replica_groups=rg)

            # ---- residents ----
            gidx = res.tile([128, GIDX_COLS], i16, tag="gidx")
            nc.sync.dma_start(out=gidx[:], in_=gidx_d[:, :])
            dloc = res.tile([128, NOPS], bf16, tag="dloc")
            nc.sync.dma_start(out=dloc[:], in_=dloc_d[:, :])
            W1 = res.tile([IN, H], bf16, tag="W1")
            nc.sync.dma_start(out=W1[:], in_=W1_d[:, :])
            W2 = res.tile([H, H], bf16, tag="W2")
            nc.sync.dma_start(out=W2[:], in_=W2_d[:, :])
            W3 = res.tile([H, 1], bf16, tag="W3")
            nc.sync.dma_start(out=W3[:], in_=W3_d[:, :])
            gbe = res.tile([128, 4], f32, tag="gbe")
            nc.sync.dma_start(out=gbe[:], in_=gbe_d[:, :])
            iden = res.tile([128, 128], bf16, tag="iden")
            nc.sync.dma_start(out=iden[:], in_=iden_d[:, :])
            iota = res.tile([128, 128], bf16, tag="iota")
            nc.sync.dma_start(out=iota[:], in_=iota_d[:, :])
            onesr = res.tile([1, 128], bf16, tag="onesr")
            nc.sync.dma_start(out=onesr[:], in_=onesr_d[:, :])
            maskc = res.tile([128, TILES], bf16, tag="maskc")
            nc.sync.dma_start(out=maskc[:], in_=maskc_d[:, :])
            degc = res.tile([128, TILES], f32, tag="degc")
            nc.sync.dma_start(out=degc[:], in_=degc_d[:, :])

            disc = res.tile([128, TILES], f32, tag="disc")
            nc.vector.reciprocal(out=disc[:], in_=degc[:])
            nc.scalar.sqrt(out=disc[:], in_=disc[:])

            def mkchunks(prefix):
                out = []
                for j, (p, a, b) in enumerate(chunks):
                    out.append(res.tile([128, b - a], bf16, tag=f"{prefix}{j}",
                                        name=f"{prefix}{j}"))
                return out
            hta = mkchunks("hta")
            htb = mkchunks("htb")
            hpre_t = [res.tile([128, 128], bf16, tag=f"hpre{t}", name=f"hpre{t}")
                      for t in range(TILES)]
            o_sb = res.tile([128, TILES], f32, tag="o_sb")
            MAXCNT = max(
                sum(1 for o in ops if o[0] == b) for b in range(NBAT))
            iorep = res.tile([128, MAXCNT, 128], bf16, tag="iorep")
            nc.vector.tensor_copy(
                out=iorep[:],
                in_=iota[:].rearrange("p (o f) -> p o f", o=1).broadcast_to(
                    [128, MAXCNT, 128]))

            def piece_of_tile(t):
                for p in range(NP):
                    if t < pstart[p + 1]:
                        return p
                raise AssertionError

            def phase_A(L, hin, Wt, AB):
                """[fused BN-apply] + project + transpose + dis-scale ->
                per-piece table shard -> per-piece AllGather."""
                for j, (p, a, b) in enumerate(chunks):
                    nt = (b - a) // 128
                    if L > 0 and AB is not None:
                        for t in range(a // 128, b // 128):
                            r = t - a // 128
                            nc.scalar.activation(
                                hin[j][:, r * 128:(r + 1) * 128], hpre_t[t][:],
                                mybir.ActivationFunctionType.Relu,
                                scale=AB[:, 0:1], bias=AB[:, 1:2])
                    sg = stg.tile([128, 4, 128], bf16, tag="stg")
                    if L < 2:
                        if L == 0:
                            xc = mtp.tile([IN, 512], bf16, tag="xc")
                            nc.sync.dma_start(out=xc[:, : b - a], in_=xT_d[:, a:b])
                            rhs = xc[:, : b - a]
                        else:
                            rhs = hin[j][:, : b - a]
                        pa = psA.tile([128, 512], f32, tag="psA")
                        nc.tensor.matmul(pa[:, : b - a], Wt[:], rhs,
                                         start=True, stop=True)
                        mt = mtp.tile([128, 512], bf16, tag="mt")
                        nc.scalar.activation(mt[:, : b - a], pa[:, : b - a],
                                             mybir.ActivationFunctionType.Copy)
                        for jj in range(nt):
                            t = (a // 128) + jj
                            pb = psB.tile([128, 128], f32, tag="psB")
                            nc.tensor.matmul(pb[:], mt[:, jj * 128:(jj + 1) * 128],
                                             iden[:], start=True, stop=True)
                            nc.vector.tensor_scalar_mul(
                                out=sg[:, jj, :], in0=pb[:],
                                scalar1=disc[:, t: t + 1])
                    else:
                        pa = psA.tile([1, 512], f32, tag="psA")
                        nc.tensor.matmul(pa[:, : b - a], Wt[:], hin[j][:, : b - a],
                                         start=True, stop=True)
                        m3c = mtp.tile([1, 512], bf16, tag="m3c")
                        nc.scalar.activation(m3c[:, : b - a], pa[:, : b - a],
                                             mybir.ActivationFunctionType.Copy)
                        for jj in range(nt):
                            t = (a // 128) + jj
                            pb = psB.tile([128, 128], f32, tag="psB")
                            nc.tensor.matmul(pb[:], m3c[:, jj * 128:(jj + 1) * 128],
                                             onesr[:], start=True, stop=True)
                            nc.vector.tensor_scalar_mul(
                                out=sg[:, jj, :], in0=pb[:],
                                scalar1=disc[:, t: t + 1])
                    # one batched DMA for the whole chunk's table rows
                    tloc0 = (a // 128) - pstart[p]
                    out_ap = tbl_loc[L][p][tloc0 * 128: tloc0 * 128 + nt * 128, :]
                    out_ap = out_ap.rearrange("(j q) h -> q j h", q=128)
                    nc.sync.dma_start(out=out_ap, in_=sg[:, :nt, :])
                    if j + 1 == NSL or chunks[j + 1][0] != p:
                        nc.gpsimd.collective_compute(
                            "AllGather", mybir.AluOpType.bypass,
                            ins=[tbl_loc[L][p][:]], outs=[tbl_full[L][p][:]],
                            replica_groups=rg)

            def phase_CD(L):
                FW = 1 if L == 2 else H     # scatter rhs feature width
                sts = None
                if L < 2:
                    sts = psST.tile([128, 2], f32, tag="sts")
                    nc.vector.memset(sts[:], 0.0)
                ops_span = {}
                for o_id, (bb, _, _) in enumerate(ops):
                    if bb not in ops_span:
                        ops_span[bb] = [o_id, 0]
                    ops_span[bb][1] += 1
                cur_g = -1
                pst = None
                for bb in range(NBAT):
                    g, p, h, size, off = batches[bb]
                    if g != cur_g:
                        cur_g = g
                        pst = psACC.tile([128, G, FW], f32, tag="acc",
                                         name=f"accL{L}g{g}")
                        nc.vector.memset(pst[:], 0.0)
                    m = msg.tile([128, NI_MAX // 128, H], bf16, tag="msg")
                    co = col_off[bb]
                    hoff = h * HWIN
                    hrows = min(HWIN, C * PIECES[p] * 128 - hoff)
                    nc.gpsimd.dma_gather(
                        out_ap=m[:, : size // 128, :],
                        in_ap=tbl_full[L][p][hoff: hoff + hrows, :],
                        idxs_ap=gidx[:, co: co + size // 16],
                        num_idxs=size, num_idxs_reg=size, elem_size=H,
                        queue_num=bb % 4)
                    o0, cnt = ops_span[bb]
                    oh = ohp.tile([128, cnt, 128], bf16, tag="oh",
                                  name=f"ohL{L}b{bb}")
                    dloc_b = dloc[:, o0: o0 + cnt].rearrange(
                        "p (o f) -> p o f", f=1).broadcast_to([128, cnt, 128])
                    nc.vector.tensor_tensor(out=oh[:], in0=iorep[:, : cnt, :],
                                            in1=dloc_b,
                                            op=mybir.AluOpType.is_equal)
                    for j in range(cnt):
                        o_id = o0 + j
                        _, col, t = ops[o_id]
                        ti = t - g * G
                        nc.tensor.matmul(pst[:, ti, :], oh[:, j, :],
                                         m[:, col, 0:FW],
                                         start=False, stop=False,
                                         skip_group_check=True)
                        if last_op[t] == o_id:
                            finish_tile(L, t, pst, ti, sts)
                if L < 2:
                    return finish_layer_stats(L, sts)
                nc.sync.dma_start(out=out_d[:, :], in_=o_sb[:])
                return None

            def finish_tile(L, t, pst, ti, sts):
                p = piece_of_tile(t)
                tt = t - pstart[p]
                if L == 2:
                    tbt = tbp.tile([128, 1], bf16, tag="tbt1", name=f"tbtL2t{t}")
                    nc.sync.dma_start(
                        out=tbt[:],
                        in_=tbl_loc[L][p][tt * 128:(tt + 1) * 128, 0:1])
                    nc.tensor.matmul(pst[:, ti, :], iden[:], tbt[:],
                                     start=False, stop=False,
                                     skip_group_check=True)
                    nc.scalar.activation(o_sb[:, t: t + 1], pst[:, ti, 0:1],
                                         mybir.ActivationFunctionType.Sigmoid,
                                         scale=disc[:, t: t + 1], bias=b3)
                    return
                tbt = tbp.tile([128, 128], bf16, tag="tbt", name=f"tbtL{L}t{t}")
                nc.sync.dma_start(out=tbt[:],
                                  in_=tbl_loc[L][p][tt * 128:(tt + 1) * 128, :])
                nc.tensor.matmul(pst[:, ti, :], iden[:], tbt[:],
                                 start=False, stop=False, skip_group_check=True)
                hp = hpp.tile([128, 128], bf16, tag="hp")
                nc.scalar.activation(hp[:], pst[:, ti, :],
                                     mybir.ActivationFunctionType.Copy,
                                     scale=disc[:, t: t + 1])
                sq = sqp.tile([128, 128], bf16, tag="sq")
                nc.scalar.activation(sq[:], hp[:],
                                     mybir.ActivationFunctionType.Square)
                nc.tensor.matmul(sts[:, 0:1], hp[:], maskc[:, t: t + 1],
                                 start=False, stop=False, skip_group_check=True)
                nc.tensor.matmul(sts[:, 1:2], sq[:], maskc[:, t: t + 1],
                                 start=False, stop=False, skip_group_check=True)
                pb = psB.tile([128, 128], f32, tag="psB")
                nc.tensor.matmul(pb[:], hp[:], iden[:], start=True, stop=True)
                nc.scalar.activation(hpre_t[t][:], pb[:],
                                     mybir.ActivationFunctionType.Copy)

            def finish_layer_stats(L, sts):
                stat = colp.tile([128, 2], f32, tag="stat")
                nc.vector.tensor_copy(out=stat[:], in_=sts[:])
                nc.sync.dma_start(out=st_in[L][:, :], in_=stat[:])
                nc.gpsimd.collective_compute(
                    "AllReduce", mybir.AluOpType.add,
                    ins=[st_in[L][:]], outs=[st_out[L][:]], replica_groups=rg)
                stg_ = colp.tile([128, 2], f32, tag="statg")
                nc.sync.dma_start(out=stg_[:], in_=st_out[L][:, :])
                mu = colp.tile([128, 4], f32, tag="mu")
                inv_n = 1.0 / float(N)
                nc.vector.tensor_scalar_mul(out=mu[:, 0:2], in0=stg_[:],
                                            scalar1=inv_n)
                nc.vector.tensor_tensor(out=mu[:, 2:3], in0=mu[:, 0:1],
                                        in1=mu[:, 0:1], op=mybir.AluOpType.mult)
                nc.vector.tensor_tensor(out=mu[:, 2:3], in0=mu[:, 1:2],
                                        in1=mu[:, 2:3],
                                        op=mybir.AluOpType.subtract)
                nc.vector.tensor_scalar_add(out=mu[:, 2:3], in0=mu[:, 2:3],
                                            scalar1=BN_EPS)
                nc.vector.reciprocal(out=mu[:, 3:4], in_=mu[:, 2:3])
                nc.scalar.sqrt(out=mu[:, 3:4], in_=mu[:, 3:4])
                AB = colp.tile([128, 2], f32, tag=f"AB{L}", name=f"AB{L}")
                gcol = gbe[:, 2 * L: 2 * L + 1]
                becol = gbe[:, 2 * L + 1: 2 * L + 2]
                nc.vector.tensor_tensor(out=AB[:, 0:1], in0=gcol, in1=mu[:, 3:4],
                                        op=mybir.AluOpType.mult)
                nc.vector.tensor_tensor(out=AB[:, 1:2], in0=mu[:, 0:1],
                                        in1=AB[:, 0:1], op=mybir.AluOpType.mult)
                nc.vector.tensor_tensor(out=AB[:, 1:2], in0=becol, in1=AB[:, 1:2],
                                        op=mybir.AluOpType.subtract)
                return AB

            # ---- run 3 layers ----
            ab_of = {}
            for L in range(3):
                hin = None if L == 0 else (htb if L == 1 else hta)
                Wt = [W1, W2, W3][L]
                phase_A(L, hin, Wt, ab_of.get(L - 1))
                ab_of[L] = phase_CD(L)

    nc.finalize()
    return nc


def make_inputs(cfg, prep, inputs, core):
    """Per-core input map."""
    N, NPC, NL, IN = cfg["N"], cfg["NPC"], cfg["NL"], cfg["IN"]
    TILES = NL // 128
    bf = ml_dtypes.bfloat16
    x = np.asarray(inputs["x"], np.float32)
    deg = prep["deg"]

    xl = np.zeros((NL, IN), np.float32)
    xl[:NPC] = x[core * NPC:(core + 1) * NPC]
    degl = np.ones(NL, np.float32)
    degl[:NPC] = deg[core * NPC:(core + 1) * NPC]
    mask = np.zeros(NL, np.float32)
    mask[:NPC] = 1.0

    gbe = np.stack([
        np.asarray(inputs["g1"], np.float32), np.asarray(inputs["be1"], np.float32),
        np.asarray(inputs["g2"], np.float32), np.asarray(inputs["be2"], np.float32),
    ], axis=1)  # (128, 4)

    return {
        "xT": xl.T.astype(bf).copy(),
        "degc": degl.reshape(TILES, 128).T.copy(),
        "maskc": mask.reshape(TILES, 128).T.astype(bf).copy(),
        "gidx": prep["gidx_data"][core],
        "dloc": prep["dloc_data"][core].astype(bf),
        "W1": np.asarray(inputs["W1"], np.float32).astype(bf),
        "W2": np.asarray(inputs["W2"], np.float32).astype(bf),
        "W3": np.asarray(inputs["W3"], np.float32).astype(bf),
        "gbe": gbe,
        "iden": np.eye(128, dtype=np.float32).astype(bf),
        "iota": np.tile(np.arange(128, dtype=np.float32), (128, 1)).astype(bf),
        "onesr": np.ones((1, 128), np.float32).astype(bf),
    }


def unshard_output(cfg, results):
    N, NPC, NL = cfg["N"], cfg["NPC"], cfg["NL"]
    TILES = NL // 128
    out = np.zeros((N, 1), np.float32)
    for c in range(C):
        o = results[c]["out"]            # (128, TILES)
        flat = o.T.reshape(NL)           # node-major
        out[c * NPC:(c + 1) * NPC, 0] = flat[:NPC]
    return out


def _ensure_axon_hooks_shim():
    """bass_utils' trace path imports antenv.axon_hooks, which this image
    lacks; register a no-op so a stray BASS_TRACE=1 can't crash the run."""
    import types
    if 'antenv.axon_hooks' in sys.modules:
        return
    try:
        import antenv
        from antenv import axon_hooks  # noqa: F401
    except ImportError:
        mod = types.ModuleType('antenv.axon_hooks')
        _hook = [None]
        mod.set_axon_ntff_profile_hook = lambda h: _hook.__setitem__(0, h)
        mod.get_axon_ntff_profile_hook = lambda: _hook[0]
        sys.modules['antenv.axon_hooks'] = mod
        try:
            antenv.axon_hooks = mod
        except Exception:
            pass


def kernel(**inputs):
    import os
    import numpy as np
    from concourse import bass_utils

    _ensure_axon_hooks_shim()
    cfg = make_cfg(full=True)
    inputs = {k: np.asarray(v) for k, v in inputs.items()}
    prep = host_prep(cfg, inputs["edge_index"])
    nc = build_graph(cfg, prep, inputs)
    in_maps = [make_inputs(cfg, prep, inputs, c) for c in range(C)]
    prev = os.environ.get("BASS_NEVER_TRACE")
    os.environ["BASS_NEVER_TRACE"] = "1"
    try:
        res = bass_utils.run_bass_kernel_spmd(nc, in_maps, list(range(C)), trace=False)
    finally:
        if prev is None:
            os.environ.pop("BASS_NEVER_TRACE", None)
        else:
            os.environ["BASS_NEVER_TRACE"] = prev
    return unshard_output(cfg, [res.results[c] for c in range(C)])
